# revision 4
# baseline (speedup 1.0000x reference)
"""Trainium2 Bass kernel for nn_K_attention_12086037971047 (v2).

out[b] = x + Km @ x,  Km = exp(-sigma*d2) with zero diagonal
       = a (.) (E @ (a (.) x)),   a_i = exp(-sigma*||x_i||^2),
         E = exp(2*sigma* x x^T)  (symmetric; its diagonal exactly
         reproduces the identity term x_i, so no correction is needed).

Design (per batch; T=2048, C=64, P=128, 16 row blocks), driven by the
fact that the Activation engine's exp over the T x T kernel is the
bottleneck (1 elem/lane/cycle at 1.2 GHz):

  - exp only the UPPER-triangle block rows of E (halves ACT work):
    e_k = exp(2s * G[rows k, cols >= 128k]), G accumulated in PSUM from
    bf16 matmuls (bf16 is 1 PE cycle/row at any output width).
  - lower blocks are recovered by transposing each row's strict-upper
    tail AFTER the exp: the first N_PE blocks per row on the PE (bf16
    transposes into PSUM + DVE copy-back), the rest with one XBAR
    dma_start_transpose per row on the otherwise idle DMA hardware.
    (The XBAR's per-16x128-tile semantics only match the simulator when
    the output has 128 partitions, so x^T itself is built on the PE.)
  - phase 2: for each target row block t, z[t] accumulates 16 matmuls
    (lhsT = stored e_j column-block or transposed et_t block, rhs =
    y = a (.) x, N=64) in a single clean PSUM group. A PSUM bank holds
    only ONE open accumulation group at a time, so z lives in two banks
    with a target->slot mapping that keeps group lifetimes disjoint and
    lets z(15) open early in the bank whose groups closed at k=12.
  - heavy software pipelining: G runs one iteration ahead of the tail
    transposes, z lags the XBAR latency, the next batch's x load /
    stats / x^T / first G rows are emitted mid-batch, and the epilogue
    (out = a (.) z) is chunked so stores drain before the final block.

Sharding: data-parallel over B: 16 batches -> 8 cores x 2 batches.
Cost-model prediction: ~53.2 us/core (baseline kernel: ~119.4 us).
"""

import numpy as np

import concourse.bass as bass
import concourse.mybir as mybir
import concourse.tile as tile
from concourse import bacc
from concourse.bass_utils import run_bass_kernel_spmd
from concourse.masks import make_identity

B, T, C = 16, 2048, 64
N_CORES = 8
B_LOC = B // N_CORES
P = 128
NB = T // P  # 16 row blocks
GCH = 8     # G chunk in blocks (8*128 = 1024 cols = 2 PSUM banks)
N_PE = 5    # strict-upper blocks per row transposed on the PE (rest: XBAR DMA)
LAG = 2     # z(t) emitted at iteration t+LAG when et_t needs the XBAR DMA
PRE_G0 = 6   # iteration of the prior batch at which the next batch's G(0) is emitted
PRE_G1 = 12
PRE_G2 = 99
PRE_LOAD = 1   # iteration at which the next batch's x load is issued
PRE_STATS = 3  # iteration at which the next batch's stats/xT quarters begin
EARLY_LAG = 3  # deeper lag for the first targets (et_0 DMA is long)
LAG_PE = 0  # z(t) lag when et_t's tail fits in the N_PE PE-transposed blocks

ROW_OFF = []
_o = 0
for _k in range(NB):
    ROW_OFF.append(_o)
    _o += NB - _k
TOT = _o  # 136 blocks in the flat upper triangle

F32 = mybir.dt.float32
BF16 = mybir.dt.bfloat16
AF = mybir.ActivationFunctionType
OP = mybir.AluOpType


def _emit(tc: tile.TileContext, x, rs, out, reps: int = 1):
    nc = tc.nc
    import contextlib

    with contextlib.ExitStack() as ctx:
        singles = ctx.enter_context(tc.tile_pool(name="singles", bufs=1))
        sb = ctx.enter_context(tc.tile_pool(name="sb", bufs=2))
        ps = ctx.enter_context(tc.tile_pool(name="ps", bufs=1, space="PSUM"))

        # --- constants (sig DMA + derived scales are emitted after the
        # first x chunk; see below) ---
        sig = singles.tile([P, 1], F32)
        neg_sig = singles.tile([P, 1], F32)
        two_sig = singles.tile([P, 1], F32)
        identf = singles.tile([P, P], F32)
        make_identity(nc, identf)
        identb = singles.tile([P, P], BF16)
        nc.vector.tensor_copy(identb, identf)

        HB = NB // 2  # prologue half, in blocks
        QB = NB // 4  # prefetched-batch xT build granularity
        COLD_CHUNKS = (QB, QB, QB, QB)  # batch-0 x chunking

        def prologue_load(b, first=False):
            # x in row layout: partition p holds rows o*128+p. Chunked so
            # downstream compute can start as soon as the first piece lands.
            x_rows = sb.tile([P, NB, C], F32, tag="x_rows", name="x_rows")
            xsrc = x[b].rearrange("(o p) c -> p o c", p=P)
            h0 = 0
            for step in COLD_CHUNKS if first else (HB, HB):
                nc.sync.dma_start(
                    x_rows[:, h0 : h0 + step, :], xsrc[:, h0 : h0 + step, :]
                )
                h0 += step
            return x_rows

        def prologue_compute(x_rows, first=False):
            sq = sb.tile([P, NB], F32, tag="sq", name="sq")
            a_t = sb.tile([P, NB], F32, tag="a_t", name="a_t")
            y_t = sb.tile([P, NB, C], BF16, tag="y_t", name="y_t")
            xT = sb.tile([C, NB, P], BF16, tag="xT", name="xT")
            if not first:
                pre_xb = sb.tile([P, NB, C], BF16, tag="xb", name="xb")
            else:
                pre_xb = None
            for h0 in range(0, NB, HB):
                hw = HB
                xsq = sb.tile([P, HB, C], F32, tag="xsq", name="xsq")
                nc.gpsimd.tensor_mul(
                    xsq, x_rows[:, h0 : h0 + hw, :], x_rows[:, h0 : h0 + hw, :]
                )
                nc.vector.tensor_reduce(
                    sq[:, h0 : h0 + hw], xsq, axis=mybir.AxisListType.X, op=OP.add
                )
                nc.scalar.activation(
                    a_t[:, h0 : h0 + hw], sq[:, h0 : h0 + hw], AF.Exp, scale=neg_sig
                )
                nc.vector.tensor_tensor(
                    y_t[:, h0 : h0 + hw, :],
                    x_rows[:, h0 : h0 + hw, :],
                    a_t[:, h0 : h0 + hw, None].to_broadcast([P, hw, C]),
                    OP.mult,
                )
            return {"a_t": a_t, "y_t": y_t, "xT": xT, "xb": pre_xb,
                    "x_rows": x_rows}

        def make_state(b, pre, first=False):
            st = {"b": b, "g_done": set()}
            a_t, y_t, xT = pre["a_t"], pre["y_t"], pre["xT"]
            # one flat tile holds the whole upper triangle; per-row views.
            # exp instructions then chunk the FLAT space (8 blocks each),
            # crossing row boundaries: 17 exps/batch instead of 22.
            e_all = sb.tile([P, TOT, P], BF16, tag="e_all", name="e_all")
            e = [
                e_all[:, ROW_OFF[k] : ROW_OFF[k] + NB - k, :] for k in range(NB)
            ]
            et = [
                sb.tile([P, NB - 1 - k, P], BF16, tag=f"et{k}", name=f"et{k}")
                for k in range(NB - 1)
            ]
            # two z PSUM tiles; targets 14,15 share tile A with 0..5 so the
            # final z writes never collide (WAR) with a pending combine read
            # of their tile. Safe: a bank holds one OPEN accumulation group
            # at a time, and groups 0..5 close long before 14 opens.
            z_a = ps.tile([P, 8, C], F32, tag="za", name="z_a")
            z_b = ps.tile([P, 8, C], F32, tag="zb", name="z_b")

            out_sb = sb.tile([P, NB, C], F32, tag="out_sb", name="out_sb")

            def z_slice(t):
                # za: targets 0..5 -> 0..5, 13 -> 6, 14 -> 7
                # zb: targets 6..12 -> 0..6, 15 -> 7
                if t < 6:
                    return z_a[:, t, :]
                if t in (13, 14):
                    return z_a[:, t - 7, :]
                if t == 15:
                    return z_b[:, 7, :]
                return z_b[:, t - 6, :]

            def z_range(t0, t1):
                if t1 <= 6:
                    return z_a[:, t0:t1, :]
                if (t0, t1) == (13, 15):
                    return z_a[:, 6:8, :]
                if (t0, t1) == (15, 16):
                    return z_b[:, 7:8, :]
                assert (t0, t1) == (6, 13)
                return z_b[:, 0:7, :]

            def g_piece(k, b0, b1):
                # G row-block k, upper-slice blocks [b0, b1) + exp
                # (cold-start path: piece-wise over one row)
                for c0 in range(b0, b1, GCH):
                    w = min(GCH, b1 - c0)
                    g = ps.tile([P, GCH, P], F32, tag="g", bufs=2, name="g")
                    for s0 in range(0, w, 4):  # matmul per PSUM bank
                        s1 = min(s0 + 4, w)
                        nc.tensor.matmul(
                            g[:, s0:s1, :],
                            lhsT=xT[:, k, :],
                            rhs=xT[:, k + c0 + s0 : k + c0 + s1, :],
                            start=True,
                            stop=True,
                            skip_group_check=True,
                        )
                    nc.scalar.activation(
                        e[k][:, c0 : c0 + w, :], g[:, :w, :], AF.Exp, scale=two_sig
                    )

            def emit_gchunk(c):
                # flat chunk c covers flat blocks [8c, 8c+w): G matmuls per
                # (row-segment x PSUM bank), one exp over the whole chunk
                if c in st["g_done"] or c * GCH >= TOT:
                    return
                st["g_done"].add(c)
                f0 = c * GCH
                w = min(GCH, TOT - f0)
                g = ps.tile([P, GCH, P], F32, tag="g", bufs=2, name="g")
                for k in range(NB):
                    s0 = max(f0, ROW_OFF[k])
                    s1 = min(f0 + w, ROW_OFF[k] + NB - k)
                    while s0 < s1:
                        # split at bank boundaries of the g tile
                        s2 = min(s1, f0 + (((s0 - f0) // 4) + 1) * 4)
                        blk = s0 - ROW_OFF[k]  # block index within row k
                        nc.tensor.matmul(
                            g[:, s0 - f0 : s2 - f0, :],
                            lhsT=xT[:, k, :],
                            rhs=xT[:, k + blk : k + blk + (s2 - s0), :],
                            start=True,
                            stop=True,
                            skip_group_check=True,
                        )
                        s0 = s2
                nc.scalar.activation(
                    e_all[:, f0 : f0 + w, :], g[:, :w, :], AF.Exp, scale=two_sig
                )

            def emit_g(k):
                # iteration hook: keep the exp stream 2 chunks ahead so every
                # row completes at-or-before its legacy iteration
                emit_gchunk(2 * k)
                emit_gchunk(2 * k + 1)

            def emit_tp(k):
                # transpose strict-upper tail of row k into et_k
                ntail = NB - 1 - k
                if ntail <= 0:
                    return
                npe = min(N_PE, ntail)
                tp = ps.tile([P, N_PE, P], BF16, tag="tp", bufs=2, name="tp")
                for i in range(npe):
                    nc.tensor.matmul(
                        tp[:, i, :],
                        lhsT=e[k][:, 1 + i, :],
                        rhs=identb,
                        is_transpose=True,
                        skip_group_check=True,
                    )
                nc.vector.tensor_copy(et[k][:, :npe, :], tp[:, :npe, :])
                if ntail > npe:
                    nc.sync.dma_start_transpose(
                        et[k][:, npe:, :], e[k][:, 1 + npe :, :]
                    )

            def emit_z(t, j0=0, j1=None):
                # z[target block t]: sources j0..j1-1 of the 16, N=64 each.
                # Stored e_j covers sources j <= t; et_t the sources above t.
                j1 = NB if j1 is None else j1
                zs = z_slice(t)
                for j in range(j0, j1):
                    lhsT = (
                        e[j][:, t - j, :] if j <= t else et[t][:, j - t - 1, :]
                    )
                    nc.tensor.matmul(
                        zs,
                        lhsT=lhsT,
                        rhs=y_t[:, j, :],
                        start=(j == j0 and j0 == 0),
                        stop=(j == j1 - 1 and j1 == NB),
                        skip_group_check=True,
                    )

            def emit_combine(t0, t1):
                # out rows blocks [t0, t1) = a (.) z, then store. The
                # penultimate store rides the ACT hwdge queue (idle after the
                # final exp) so the last store never queues behind it on SP.
                nc.vector.tensor_tensor(
                    out_sb[:, t0:t1, :],
                    z_range(t0, t1),
                    a_t[:, t0:t1, None].to_broadcast([P, t1 - t0, C]),
                    OP.mult,
                )
                eng = nc.scalar if (t0, t1) == (13, 15) else nc.sync
                eng.dma_start(
                    out[b].rearrange("(o p) c -> p o c", p=P)[:, t0:t1, :],
                    out_sb[:, t0:t1, :],
                )

            st.update(
                emit_g=emit_g, emit_tp=emit_tp, emit_z=emit_z,
                emit_combine=emit_combine, g_piece=g_piece,
            )
            return st

        # z(t)'s lag: 2 iterations when et_t needs the XBAR DMA, 1 when the
        # tail is fully PE-transposed (no DMA latency to hide)
        def z_lag(t):
            ntail = NB - 1 - t
            if ntail > N_PE:
                return max(LAG, EARLY_LAG - t)
            return 0 if ntail <= 2 else LAG_PE

        zsched = {}
        done_k = {}
        for t in range(NB):
            kk = NB - 2 if t == NB - 1 else min(t + z_lag(t), NB - 1)
            zsched.setdefault(kk, []).append(t)
            done_k[t] = kk
        # finer combines near the end shrink the serial tail; each fires once
        # every target in its range has completed
        combines = {}
        for rng in [(0, 6), (6, 13), (13, 15), (15, 16)]:
            combines.setdefault(max(done_k[t] for t in range(*rng)), []).append(rng)

        def xT_quarter(pre, q0):
            xb, xT, x_rows = pre["xb"], pre["xT"], pre["x_rows"]
            nc.vector.tensor_copy(
                xb[:, q0 : q0 + QB, :], x_rows[:, q0 : q0 + QB, :]
            )
            tpx = ps.tile([C, QB, P], BF16, tag="tp", bufs=2, name="tpx")
            for i in range(QB):
                nc.tensor.matmul(
                    tpx[:, i, :],
                    lhsT=xb[:, q0 + i, :],
                    rhs=identb,
                    is_transpose=True,
                    skip_group_check=True,
                )
            nc.vector.tensor_copy(xT[:, q0 : q0 + QB, :], tpx)

        def cold_xT(st, x_rows, xT):
            # build xT via PE transposes + DVE copies per quarter, firing each
            # G(0) piece as soon as its quarter of xT lands (x DMA latency is
            # the critical path; PE/DVE idle)
            st["g_done"].update((0, 1))
            q0 = 0
            for w in COLD_CHUNKS:
                # shares the tp tag (slot grows to fit); its lifetime ends
                # before the first tp use
                tpx = ps.tile([C, QB, P], F32, tag="tp", bufs=2, name="tpx")
                for i in range(w):
                    nc.tensor.matmul(
                        tpx[:, i, :],
                        lhsT=x_rows[:, q0 + i, :],
                        rhs=identf,
                        is_transpose=True,
                        skip_group_check=True,
                    )
                nc.vector.tensor_copy(xT[:, q0 : q0 + w, :], tpx[:, :w, :])
                st["g_piece"](0, q0, q0 + w)
                q0 += w

        nc.sync.dma_start(sig, rs[:].to_broadcast([P, 1]))
        nc.scalar.mul(neg_sig, sig, -1.0)
        nc.scalar.mul(two_sig, sig, 2.0)

        def rep_body(n_reps=1):
            bs = [bb for _ in range(n_reps) for bb in range(B_LOC)]
            nxt_load = prologue_load(bs[0], first=True)
            pre0 = prologue_compute(nxt_load, first=True)
            st = make_state(bs[0], pre0, first=True)
            cold_xT(st, nxt_load, pre0["xT"])
            for bi, b in enumerate(bs):
                nxt_st = None
                st["emit_g"](0)
                for k in range(NB):
                    st["emit_g"](k + 1) if k + 1 < NB else None
                    st["emit_tp"](k)
                    if k == PRE_LOAD and bi + 1 < len(bs):
                        nxt_load = prologue_load(bs[bi + 1])
                    if k == PRE_STATS and bi + 1 < len(bs):
                        nxt_pre = prologue_compute(nxt_load)
                        nxt_st = make_state(bs[bi + 1], nxt_pre)
                    if PRE_STATS <= k <= PRE_STATS + 3 and bi + 1 < len(bs):
                        xT_quarter(nxt_pre, (k - PRE_STATS) * QB)
                    if k >= PRE_G0 and nxt_st is not None:
                        # stream the next batch's G/exp chunks as soon as the
                        # prior batch's late iterations have slack
                        nxt_st["emit_g"](k - PRE_G0)
                    for t in zsched.get(k, []):
                        st["emit_z"](t)
                    for rng in combines.get(k, []):
                        st["emit_combine"](*rng)
                if nxt_st is not None:
                    st = nxt_st

        # reps live in a HARDWARE loop: the program (and NEFF) stays the same
        # size regardless of reps, each iteration re-runs the full pipeline on
        # device, so marginal wall-clock per rep measures device time rather
        # than host-side lowering of an unrolled program. K rep-bodies are
        # emitted per loop iteration (software-pipelined into one another) to
        # amortize the loop's all-engine barrier and the cold-start latency.
        K_UNROLL = 4
        n_loop, n_tail = divmod(reps, K_UNROLL)
        if n_loop == 1:
            n_loop, n_tail = 0, reps
        if n_loop:
            with tc.For_i(0, n_loop):
                rep_body(K_UNROLL)
        if n_tail:
            rep_body(n_tail)


def build(reps: int = 1):
    nc = bacc.Bacc("TRN2", target_bir_lowering=False)
    x = nc.dram_tensor("x", [B_LOC, T, C], F32, kind="ExternalInput")
    rs = nc.dram_tensor("r_sigma", [1], F32, kind="ExternalInput")
    out = nc.dram_tensor("out", [B_LOC, T, C], F32, kind="ExternalOutput")
    with tile.TileContext(nc) as tc:
        _emit(tc, x, rs, out, reps=reps)
    nc.compile()
    return nc


_NC = None


def _get_nc():
    global _NC
    if _NC is None:
        _NC = build()
    return _NC


def kernel(x: np.ndarray, r_sigma: np.ndarray) -> np.ndarray:
    x = np.ascontiguousarray(x, dtype=np.float32)
    r_sigma = np.ascontiguousarray(r_sigma, dtype=np.float32)
    nc = _get_nc()
    in_maps = [
        {"x": x[i * B_LOC : (i + 1) * B_LOC], "r_sigma": r_sigma}
        for i in range(N_CORES)
    ]
    res = run_bass_kernel_spmd(nc, in_maps, core_ids=list(range(N_CORES)))
    return np.concatenate([r["out"] for r in res.results], axis=0)



# revision 7
# speedup vs baseline: 2.3886x; 2.3886x over previous
"""Trainium2 Bass kernel for nn_K_attention_12086037971047 (v2).

out[b] = x + Km @ x,  Km = exp(-sigma*d2) with zero diagonal
       = a (.) (E @ (a (.) x)),   a_i = exp(-sigma*||x_i||^2),
         E = exp(2*sigma* x x^T)  (symmetric; its diagonal exactly
         reproduces the identity term x_i, so no correction is needed).

Design (per batch; T=2048, C=64, P=128, 16 row blocks), driven by the
fact that the Activation engine's exp over the T x T kernel is the
bottleneck (1 elem/lane/cycle at 1.2 GHz):

  - exp only the UPPER-triangle block rows of E (halves ACT work):
    e_k = exp(2s * G[rows k, cols >= 128k]), G accumulated in PSUM from
    bf16 matmuls (bf16 is 1 PE cycle/row at any output width).
  - lower blocks are recovered by transposing each row's strict-upper
    tail AFTER the exp: the first N_PE blocks per row on the PE (bf16
    transposes into PSUM + DVE copy-back), the rest with one XBAR
    dma_start_transpose per row on the otherwise idle DMA hardware.
    (The XBAR's per-16x128-tile semantics only match the simulator when
    the output has 128 partitions, so x^T itself is built on the PE.)
  - phase 2: for each target row block t, z[t] accumulates 16 matmuls
    (lhsT = stored e_j column-block or transposed et_t block, rhs =
    y = a (.) x, N=64) in a single clean PSUM group. A PSUM bank holds
    only ONE open accumulation group at a time, so z lives in two banks
    with a target->slot mapping that keeps group lifetimes disjoint and
    lets z(15) open early in the bank whose groups closed at k=12.
  - heavy software pipelining: G runs one iteration ahead of the tail
    transposes, z lags the XBAR latency, the next batch's x load /
    stats / x^T / first G rows are emitted mid-batch, and the epilogue
    (out = a (.) z) is chunked so stores drain before the final block.

Sharding: data-parallel over B: 16 batches -> 8 cores x 2 batches.
Cost-model prediction: ~53.2 us/core (baseline kernel: ~119.4 us).
"""

import numpy as np

import concourse.bass as bass
import concourse.mybir as mybir
import concourse.tile as tile
from concourse import bacc
from concourse.bass_utils import run_bass_kernel_spmd
from concourse.masks import make_identity

B, T, C = 16, 2048, 64
N_CORES = 8
B_LOC = B // N_CORES
P = 128
NB = T // P  # 16 row blocks
GCH = 8     # G chunk in blocks (8*128 = 1024 cols = 2 PSUM banks)
N_PE = 5    # strict-upper blocks per row transposed on the PE (rest: XBAR DMA)
LAG = 2     # z(t) emitted at iteration t+LAG when et_t needs the XBAR DMA
PRE_G0 = 6   # iteration of the prior batch at which the next batch's G(0) is emitted
PRE_G1 = 12
PRE_G2 = 99
K_UNROLL = 1   # rep bodies emitted (and pipelined) per hardware-loop iteration
PRE_LOAD = 1   # iteration at which the next batch's x load is issued
PRE_STATS = 3  # iteration at which the next batch's stats/xT quarters begin
EARLY_LAG = 3  # deeper lag for the first targets (et_0 DMA is long)
LAG_PE = 0  # z(t) lag when et_t's tail fits in the N_PE PE-transposed blocks

ROW_OFF = []
_o = 0
for _k in range(NB):
    ROW_OFF.append(_o)
    _o += NB - _k
TOT = _o  # 136 blocks in the flat upper triangle

F32 = mybir.dt.float32
BF16 = mybir.dt.bfloat16
AF = mybir.ActivationFunctionType
OP = mybir.AluOpType


def _emit(tc: tile.TileContext, x, rs, out, reps: int = 1):
    nc = tc.nc
    import contextlib

    with contextlib.ExitStack() as ctx:
        singles = ctx.enter_context(tc.tile_pool(name="singles", bufs=1))
        sb = ctx.enter_context(tc.tile_pool(name="sb", bufs=2))
        ps = ctx.enter_context(tc.tile_pool(name="ps", bufs=1, space="PSUM"))

        # --- constants (sig DMA + derived scales are emitted after the
        # first x chunk; see below) ---
        sig = singles.tile([P, 1], F32)
        neg_sig = singles.tile([P, 1], F32)
        two_sig = singles.tile([P, 1], F32)
        identf = singles.tile([P, P], F32)
        make_identity(nc, identf)
        identb = singles.tile([P, P], BF16)
        nc.vector.tensor_copy(identb, identf)

        HB = NB // 2  # prologue half, in blocks
        QB = NB // 4  # prefetched-batch xT build granularity
        COLD_CHUNKS = (QB, QB, QB, QB)  # batch-0 x chunking

        def prologue_load(b, first=False):
            # x in row layout: partition p holds rows o*128+p. Chunked so
            # downstream compute can start as soon as the first piece lands.
            x_rows = sb.tile([P, NB, C], F32, tag="x_rows", name="x_rows")
            xsrc = x[b].rearrange("(o p) c -> p o c", p=P)
            h0 = 0
            for step in COLD_CHUNKS if first else (HB, HB):
                nc.sync.dma_start(
                    x_rows[:, h0 : h0 + step, :], xsrc[:, h0 : h0 + step, :]
                )
                h0 += step
            return x_rows

        def prologue_compute(x_rows, first=False):
            sq = sb.tile([P, NB], F32, tag="sq", name="sq")
            a_t = sb.tile([P, NB], F32, tag="a_t", name="a_t")
            y_t = sb.tile([P, NB, C], BF16, tag="y_t", name="y_t")
            xT = sb.tile([C, NB, P], BF16, tag="xT", name="xT")
            if not first:
                pre_xb = sb.tile([P, NB, C], BF16, tag="xb", name="xb")
            else:
                pre_xb = None
            for h0 in range(0, NB, HB):
                hw = HB
                xsq = sb.tile([P, HB, C], F32, tag="xsq", name="xsq")
                nc.gpsimd.tensor_mul(
                    xsq, x_rows[:, h0 : h0 + hw, :], x_rows[:, h0 : h0 + hw, :]
                )
                nc.vector.tensor_reduce(
                    sq[:, h0 : h0 + hw], xsq, axis=mybir.AxisListType.X, op=OP.add
                )
                nc.scalar.activation(
                    a_t[:, h0 : h0 + hw], sq[:, h0 : h0 + hw], AF.Exp, scale=neg_sig
                )
                nc.vector.tensor_tensor(
                    y_t[:, h0 : h0 + hw, :],
                    x_rows[:, h0 : h0 + hw, :],
                    a_t[:, h0 : h0 + hw, None].to_broadcast([P, hw, C]),
                    OP.mult,
                )
            return {"a_t": a_t, "y_t": y_t, "xT": xT, "xb": pre_xb,
                    "x_rows": x_rows}

        def make_state(b, pre, first=False):
            st = {"b": b, "g_done": set()}
            a_t, y_t, xT = pre["a_t"], pre["y_t"], pre["xT"]
            # one flat tile holds the whole upper triangle; per-row views.
            # exp instructions then chunk the FLAT space (8 blocks each),
            # crossing row boundaries: 17 exps/batch instead of 22.
            e_all = sb.tile([P, TOT, P], BF16, tag="e_all", name="e_all")
            e = [
                e_all[:, ROW_OFF[k] : ROW_OFF[k] + NB - k, :] for k in range(NB)
            ]
            et = [
                sb.tile([P, NB - 1 - k, P], BF16, tag=f"et{k}", name=f"et{k}")
                for k in range(NB - 1)
            ]
            # two z PSUM tiles; targets 14,15 share tile A with 0..5 so the
            # final z writes never collide (WAR) with a pending combine read
            # of their tile. Safe: a bank holds one OPEN accumulation group
            # at a time, and groups 0..5 close long before 14 opens.
            z_a = ps.tile([P, 8, C], F32, tag="za", name="z_a")
            z_b = ps.tile([P, 8, C], F32, tag="zb", name="z_b")

            out_sb = sb.tile([P, NB, C], F32, tag="out_sb", name="out_sb")

            def z_slice(t):
                # za: targets 0..5 -> 0..5, 13 -> 6, 14 -> 7
                # zb: targets 6..12 -> 0..6, 15 -> 7
                if t < 6:
                    return z_a[:, t, :]
                if t in (13, 14):
                    return z_a[:, t - 7, :]
                if t == 15:
                    return z_b[:, 7, :]
                return z_b[:, t - 6, :]

            def z_range(t0, t1):
                if t1 <= 6:
                    return z_a[:, t0:t1, :]
                if (t0, t1) == (13, 15):
                    return z_a[:, 6:8, :]
                if (t0, t1) == (15, 16):
                    return z_b[:, 7:8, :]
                assert (t0, t1) == (6, 13)
                return z_b[:, 0:7, :]

            def g_piece(k, b0, b1):
                # G row-block k, upper-slice blocks [b0, b1) + exp
                # (cold-start path: piece-wise over one row)
                for c0 in range(b0, b1, GCH):
                    w = min(GCH, b1 - c0)
                    g = ps.tile([P, GCH, P], F32, tag="g", bufs=2, name="g")
                    for s0 in range(0, w, 4):  # matmul per PSUM bank
                        s1 = min(s0 + 4, w)
                        nc.tensor.matmul(
                            g[:, s0:s1, :],
                            lhsT=xT[:, k, :],
                            rhs=xT[:, k + c0 + s0 : k + c0 + s1, :],
                            start=True,
                            stop=True,
                            skip_group_check=True,
                        )
                    nc.scalar.activation(
                        e[k][:, c0 : c0 + w, :], g[:, :w, :], AF.Exp, scale=two_sig
                    )

            def emit_gchunk(c):
                # flat chunk c covers flat blocks [8c, 8c+w): G matmuls per
                # (row-segment x PSUM bank), one exp over the whole chunk
                if c in st["g_done"] or c * GCH >= TOT:
                    return
                st["g_done"].add(c)
                f0 = c * GCH
                w = min(GCH, TOT - f0)
                g = ps.tile([P, GCH, P], F32, tag="g", bufs=2, name="g")
                for k in range(NB):
                    s0 = max(f0, ROW_OFF[k])
                    s1 = min(f0 + w, ROW_OFF[k] + NB - k)
                    while s0 < s1:
                        # split at bank boundaries of the g tile
                        s2 = min(s1, f0 + (((s0 - f0) // 4) + 1) * 4)
                        blk = s0 - ROW_OFF[k]  # block index within row k
                        nc.tensor.matmul(
                            g[:, s0 - f0 : s2 - f0, :],
                            lhsT=xT[:, k, :],
                            rhs=xT[:, k + blk : k + blk + (s2 - s0), :],
                            start=True,
                            stop=True,
                            skip_group_check=True,
                        )
                        s0 = s2
                nc.scalar.activation(
                    e_all[:, f0 : f0 + w, :], g[:, :w, :], AF.Exp, scale=two_sig
                )

            def emit_g(k):
                # iteration hook: keep the exp stream 2 chunks ahead so every
                # row completes at-or-before its legacy iteration
                emit_gchunk(2 * k)
                emit_gchunk(2 * k + 1)

            def emit_tp(k):
                # transpose strict-upper tail of row k into et_k
                ntail = NB - 1 - k
                if ntail <= 0:
                    return
                npe = min(N_PE, ntail)
                tp = ps.tile([P, N_PE, P], BF16, tag="tp", bufs=2, name="tp")
                for i in range(npe):
                    nc.tensor.matmul(
                        tp[:, i, :],
                        lhsT=e[k][:, 1 + i, :],
                        rhs=identb,
                        is_transpose=True,
                        skip_group_check=True,
                    )
                nc.vector.tensor_copy(et[k][:, :npe, :], tp[:, :npe, :])
                if ntail > npe:
                    nc.sync.dma_start_transpose(
                        et[k][:, npe:, :], e[k][:, 1 + npe :, :]
                    )

            def emit_z(t, j0=0, j1=None):
                # z[target block t]: sources j0..j1-1 of the 16, N=64 each.
                # Stored e_j covers sources j <= t; et_t the sources above t.
                j1 = NB if j1 is None else j1
                zs = z_slice(t)
                for j in range(j0, j1):
                    lhsT = (
                        e[j][:, t - j, :] if j <= t else et[t][:, j - t - 1, :]
                    )
                    nc.tensor.matmul(
                        zs,
                        lhsT=lhsT,
                        rhs=y_t[:, j, :],
                        start=(j == j0 and j0 == 0),
                        stop=(j == j1 - 1 and j1 == NB),
                        skip_group_check=True,
                    )

            def emit_combine(t0, t1):
                # out rows blocks [t0, t1) = a (.) z, then store. The
                # penultimate store rides the ACT hwdge queue (idle after the
                # final exp) so the last store never queues behind it on SP.
                nc.vector.tensor_tensor(
                    out_sb[:, t0:t1, :],
                    z_range(t0, t1),
                    a_t[:, t0:t1, None].to_broadcast([P, t1 - t0, C]),
                    OP.mult,
                )
                eng = nc.scalar if (t0, t1) == (13, 15) else nc.sync
                eng.dma_start(
                    out[b].rearrange("(o p) c -> p o c", p=P)[:, t0:t1, :],
                    out_sb[:, t0:t1, :],
                )

            st.update(
                emit_g=emit_g, emit_tp=emit_tp, emit_z=emit_z,
                emit_combine=emit_combine, g_piece=g_piece,
            )
            return st

        # z(t)'s lag: 2 iterations when et_t needs the XBAR DMA, 1 when the
        # tail is fully PE-transposed (no DMA latency to hide)
        def z_lag(t):
            ntail = NB - 1 - t
            if ntail > N_PE:
                return max(LAG, EARLY_LAG - t)
            return 0 if ntail <= 2 else LAG_PE

        zsched = {}
        done_k = {}
        for t in range(NB):
            kk = NB - 2 if t == NB - 1 else min(t + z_lag(t), NB - 1)
            zsched.setdefault(kk, []).append(t)
            done_k[t] = kk
        # finer combines near the end shrink the serial tail; each fires once
        # every target in its range has completed
        combines = {}
        for rng in [(0, 6), (6, 13), (13, 15), (15, 16)]:
            combines.setdefault(max(done_k[t] for t in range(*rng)), []).append(rng)

        def xT_quarter(pre, q0):
            xb, xT, x_rows = pre["xb"], pre["xT"], pre["x_rows"]
            nc.vector.tensor_copy(
                xb[:, q0 : q0 + QB, :], x_rows[:, q0 : q0 + QB, :]
            )
            tpx = ps.tile([C, QB, P], BF16, tag="tp", bufs=2, name="tpx")
            for i in range(QB):
                nc.tensor.matmul(
                    tpx[:, i, :],
                    lhsT=xb[:, q0 + i, :],
                    rhs=identb,
                    is_transpose=True,
                    skip_group_check=True,
                )
            nc.vector.tensor_copy(xT[:, q0 : q0 + QB, :], tpx)

        def cold_xT(st, x_rows, xT):
            # build xT via PE transposes + DVE copies per quarter, firing each
            # G(0) piece as soon as its quarter of xT lands (x DMA latency is
            # the critical path; PE/DVE idle)
            st["g_done"].update((0, 1))
            q0 = 0
            for w in COLD_CHUNKS:
                # shares the tp tag (slot grows to fit); its lifetime ends
                # before the first tp use
                tpx = ps.tile([C, QB, P], F32, tag="tp", bufs=2, name="tpx")
                for i in range(w):
                    nc.tensor.matmul(
                        tpx[:, i, :],
                        lhsT=x_rows[:, q0 + i, :],
                        rhs=identf,
                        is_transpose=True,
                        skip_group_check=True,
                    )
                nc.vector.tensor_copy(xT[:, q0 : q0 + w, :], tpx[:, :w, :])
                st["g_piece"](0, q0, q0 + w)
                q0 += w

        nc.sync.dma_start(sig, rs[:].to_broadcast([P, 1]))
        nc.scalar.mul(neg_sig, sig, -1.0)
        nc.scalar.mul(two_sig, sig, 2.0)

        def rep_body(n_reps=1):
            bs = [bb for _ in range(n_reps) for bb in range(B_LOC)]
            nxt_load = prologue_load(bs[0], first=True)
            pre0 = prologue_compute(nxt_load, first=True)
            st = make_state(bs[0], pre0, first=True)
            cold_xT(st, nxt_load, pre0["xT"])
            for bi, b in enumerate(bs):
                nxt_st = None
                st["emit_g"](0)
                for k in range(NB):
                    st["emit_g"](k + 1) if k + 1 < NB else None
                    st["emit_tp"](k)
                    if k == PRE_LOAD and bi + 1 < len(bs):
                        nxt_load = prologue_load(bs[bi + 1])
                    if k == PRE_STATS and bi + 1 < len(bs):
                        nxt_pre = prologue_compute(nxt_load)
                        nxt_st = make_state(bs[bi + 1], nxt_pre)
                    if PRE_STATS <= k <= PRE_STATS + 3 and bi + 1 < len(bs):
                        xT_quarter(nxt_pre, (k - PRE_STATS) * QB)
                    if k >= PRE_G0 and nxt_st is not None:
                        # stream the next batch's G/exp chunks as soon as the
                        # prior batch's late iterations have slack
                        nxt_st["emit_g"](k - PRE_G0)
                    for t in zsched.get(k, []):
                        st["emit_z"](t)
                    for rng in combines.get(k, []):
                        st["emit_combine"](*rng)
                if nxt_st is not None:
                    st = nxt_st

        # reps live in a HARDWARE loop: the program (and NEFF) stays the same
        # size regardless of reps, each iteration re-runs the full pipeline on
        # device, so marginal wall-clock per rep measures device time rather
        # than host-side lowering of an unrolled program. K rep-bodies are
        # emitted per loop iteration (software-pipelined into one another) to
        # amortize the loop's all-engine barrier and the cold-start latency.
        n_loop, n_tail = divmod(reps, K_UNROLL)
        if n_loop == 1:
            n_loop, n_tail = 0, reps
        if n_loop:
            with tc.For_i(0, n_loop):
                rep_body(K_UNROLL)
        if n_tail:
            rep_body(n_tail)


def build(reps: int = 1):
    nc = bacc.Bacc("TRN2", target_bir_lowering=False)
    x = nc.dram_tensor("x", [B_LOC, T, C], F32, kind="ExternalInput")
    rs = nc.dram_tensor("r_sigma", [1], F32, kind="ExternalInput")
    out = nc.dram_tensor("out", [B_LOC, T, C], F32, kind="ExternalOutput")
    with tile.TileContext(nc) as tc:
        _emit(tc, x, rs, out, reps=reps)
    nc.compile()
    return nc


_NC = None


def _get_nc():
    global _NC
    if _NC is None:
        _NC = build()
    return _NC


def kernel(x: np.ndarray, r_sigma: np.ndarray) -> np.ndarray:
    x = np.ascontiguousarray(x, dtype=np.float32)
    r_sigma = np.ascontiguousarray(r_sigma, dtype=np.float32)
    nc = _get_nc()
    in_maps = [
        {"x": x[i * B_LOC : (i + 1) * B_LOC], "r_sigma": r_sigma}
        for i in range(N_CORES)
    ]
    res = run_bass_kernel_spmd(nc, in_maps, core_ids=list(range(N_CORES)))
    return np.concatenate([r["out"] for r in res.results], axis=0)

